# revision 5
# baseline (speedup 1.0000x reference)
"""SSIM-pyramid loss kernel v2 for 8 Trainium2 NeuronCores (Bass/Tile).

Math identical to v1: per level, loss_l = 2 - 2*mean(sig12/(std1*std2)) from
5 box maps.  v2 removes BOTH the serial downsample spine and all DMA
transposes: every pyramid level's vertical filtering is a composed banded
matrix applied directly to the level-0 band (V_l = prod of bicubic verticals,
BV_l = V_l @ box_v), and every horizontal pass is a matmul whose stationary
operand is the transposed-by-construction output of the vertical pass
(stationary = data chunk -> transposed output, no DMA transpose needed).

All pyramid levels become independent matmul chains off the level-0 band, so
the TensorE runs dense (stays at the warm 2.4 GHz HAM state) and all pointwise
work runs on SBUF fp16 behind it.

Distribution: batch b = core//4, row-band i = core%4 (128 rows of L0 per
core), per-core partial sums combined on the host (same as v1).
"""

import sys

sys.path.insert(0, "/opt/trn_rl_repo")

import numpy as np

import concourse.bass as bass  # noqa: E402
import concourse.mybir as mybir  # noqa: E402
import concourse.tile as tile  # noqa: E402
from concourse import bacc  # noqa: E402
from concourse.bass_utils import run_bass_kernel_spmd  # noqa: E402

F32 = mybir.dt.float32
FP16 = mybir.dt.float16
NP16 = np.float16
AluOp = mybir.AluOpType
ActFn = mybir.ActivationFunctionType

WS, PAD = 11, 5
BIC = np.array([-0.09375, 0.59375, 0.59375, -0.09375])
PYR_W = (0.2, 0.4, 0.6, 0.8)
H = [512, 256, 128, 64]
R = [128, 64, 32, 16]
NK = [222, 110, 54, 26]
NKP = [222, 112, 64, 32]
SCL = 1.0 / 128.0  # map scale: m-maps * SCL, z-maps * SCL^2 (cancels in corr)

# vmat column layout: [Av0 | V1 | V2 | V3 | BV1 | BV2 | BV3]
# V_l sliced host-side to exactly the z-box rows (R_l+10), 16-aligned slots
NKZ = [138, 74, 42, 26]
VOFF = [0, 128, 208, 256, 288, 352, 384]
VW = 400
# xpack: [ximg(1024) | vmat(400) | avz(368)] fused so each input DMA moves
# 3584-byte rows (small-row DMAs were 6x slower on the queue)
VMO = 1024
AZO = 1424
XB = 1792   # tail rows 128:222 of the band live in cols XB.. of the same tile
XPW = 3584
# mdeep / zboxd packed tile layout: L1 [0:64,0:256] L2 [0:32,256:384] L3 [0:16,384:448]
DOFF = {1: 0, 2: 256, 3: 384}


# ----------------------------------------------------------------------------
# geometry + host matrices
# ----------------------------------------------------------------------------
def _lr_ranges(i):
    lr = [None] * 4
    s3 = 16 * i
    lr[3] = (s3 - PAD, s3 + 16 + PAD)
    for l in (2, 1, 0):
        s = R[l] * i
        box = (s - PAD, s + R[l] + PAD)
        a1, b1 = lr[l + 1]
        ds = (2 * a1 - 1, 2 * (b1 - 1) + 2 + 1)
        lr[l] = (min(box[0], ds[0]), max(box[1], ds[1]))
    return lr


def _build_v(i):
    """vmat [222, 448] fp16 and avz blocks + z row offsets ob[l]."""
    lr = _lr_ranges(i)
    dv = []
    for l in range(3):
        a, an = lr[l][0], lr[l + 1][0]
        M = np.zeros((NK[l], NKP[l + 1]))
        for m in range(NK[l + 1]):
            j = an + m
            if j < 0 or j >= H[l + 1]:
                continue
            for t in range(4):
                src = min(max(2 * j - 1 + t, 0), H[l] - 1)
                M[src - a, m] += BIC[t]
        dv.append(M)
    av = []
    for l in range(4):
        a, s = lr[l][0], R[l] * i
        M = np.zeros((NK[l], R[l]))
        for k in range(NK[l]):
            for m in range(R[l]):
                if abs((a + k) - (s + m)) <= PAD:
                    M[k, m] = 1.0
        av.append(M)
    V = [np.eye(222)]
    for l in range(3):
        V.append(V[l][:, : NK[l]] @ dv[l])
    BV = [av[0]] + [V[l][:, : NK[l]] @ av[l] for l in (1, 2, 3)]
    ob = []
    avz = []
    for l in range(4):
        o = (R[l] * i - PAD) - lr[l][0]
        ob.append(o)
        avz.append(av[l][o : o + R[l] + 2 * PAD, :])
    # band-row permutation: z-box rows (ob0..ob0+138) land at partition 0 of
    # xa, with the 10-row tail at partition 0 of xb
    o0 = ob[0]
    perm = list(range(o0, o0 + 128)) + list(range(o0 + 128, 222)) + list(range(o0))
    vmat = np.zeros((222, VW))
    vmat[:, 0:128] = av[0]
    for l in (1, 2, 3):  # deep V sliced to exactly the z-box rows
        vmat[:, VOFF[l] : VOFF[l] + NKZ[l]] = V[l][:, ob[l] : ob[l] + NKZ[l]]
    for l, w in ((1, 64), (2, 32), (3, 16)):
        vmat[:, VOFF[3 + l] : VOFF[3 + l] + w] = BV[l]
    vmat = vmat[perm]
    # avz block at AZOFF: avz0a [0:128,+0:128], avz0b [0:10,+128:256],
    # avz1 [0:74,+256:320], avz2 [0:42,+320:352], avz3 [0:26,+352:368]
    azp = np.zeros((128, 368))
    azp[0:128, 0:128] = avz[0][0:128]
    azp[0:10, 128:256] = avz[0][128:138]
    azp[0:74, 256:320] = avz[1]
    azp[0:42, 320:352] = avz[2]
    azp[0:26, 352:368] = avz[3]
    return vmat.astype(NP16), azp.astype(NP16), perm


def _build_h():
    """hpack [128, HP] fp16 + block offset registry {name: (lo, hi, off)}."""
    Bh = []
    for l in range(4):
        M = np.zeros((H[l], H[l]))
        for w in range(H[l]):
            M[w, max(0, w - PAD) : min(H[l], w + PAD + 1)] = 1.0
        Bh.append(M)
    dh = []
    for l in range(3):
        M = np.zeros((H[l], H[l + 1]))
        for j in range(H[l + 1]):
            for t in range(4):
                src = min(max(2 * j - 1 + t, 0), H[l] - 1)
                M[src, j] += BIC[t]
        dh.append(M)
    Hm = [np.eye(512)]
    for l in range(3):
        Hm.append(Hm[l] @ dh[l])
    BH = [Bh[0]] + [Hm[l] @ Bh[l] for l in (1, 2, 3)]

    cols = []
    reg = {}
    off = [0]

    def add(name, M, r0, r1, full_to=None):
        rows = r1 - r0
        sub = M[r0:r1]
        nz = np.nonzero(np.any(sub != 0, axis=0))[0]
        lo, hi = int(nz[0]), int(nz[-1]) + 1
        if full_to is not None:
            lo, hi = 0, full_to
        blk = np.zeros((128, hi - lo))
        blk[0:rows, :] = (
            sub[:, lo:hi]
            if hi <= M.shape[1]
            else np.pad(sub[:, lo:], ((0, 0), (0, hi - M.shape[1])))
        )
        reg[name] = (lo, hi, off[0])
        off[0] += hi - lo
        cols.append(blk)

    for c in range(4):
        r0, r1 = 128 * c, 128 * c + 128
        add(f"bh0_{c}", Bh[0], r0, r1, full_to=512 if c == 0 else None)
        add(f"hm1_{c}", Hm[1], r0, r1, full_to=256 if c == 0 else None)
        add(f"hm2_{c}", Hm[2], r0, r1, full_to=128 if c == 0 else None)
        add(f"hm3_{c}", Hm[3], r0, r1, full_to=64 if c == 0 else None)
        add(f"BH1_{c}", BH[1], r0, r1, full_to=448 if c == 0 else None)
        add(f"BH2_{c}", BH[2], r0, r1, full_to=128 if c == 0 else None)
        add(f"BH3_{c}", BH[3], r0, r1, full_to=64 if c == 0 else None)
    for k in range(2):
        add(f"bh1_{k}", Bh[1], 128 * k, 128 * k + 128, full_to=448 if k == 0 else None)
    add("bh2", Bh[2], 0, 128)
    add("bh3", Bh[3], 0, 64)
    hp = np.concatenate(cols, axis=1)
    return hp.astype(NP16), reg


def _band_slices(img1, img2, b, i, perm):
    a, e = _lr_ranges(i)[0]
    out = np.zeros((NK[0], 1024), np.float32)
    lo, hi = max(a, 0), min(e, 512)
    out[lo - a : hi - a, 0:512] = img1[b, 0, lo:hi, :]
    out[lo - a : hi - a, 512:1024] = img2[b, 0, lo:hi, :]
    return out[perm].astype(NP16)


def _xpack(img1, img2, b, i, vmat, azp, perm):
    band = _band_slices(img1, img2, b, i, perm)
    xp = np.zeros((128, XPW), NP16)
    xp[:, 0:VMO] = band[0:128]
    xp[:, VMO : VMO + VW] = vmat[0:128]
    xp[0:128, AZO : AZO + 368] = azp
    xp[0:94, XB : XB + VMO] = band[128:222]
    xp[0:94, XB + VMO : XB + VMO + VW] = vmat[128:222]
    return xp


# ----------------------------------------------------------------------------
# device program
# ----------------------------------------------------------------------------
_HREG = None  # set by build_program


def build_program(hreg, hp_cols):
    nc = bacc.Bacc("TRN2", target_bir_lowering=False)

    xpk = nc.dram_tensor("xpk", [128, XPW], FP16, kind="ExternalInput")
    hpack = nc.dram_tensor("hpack", [128, hp_cols], FP16, kind="ExternalInput")
    outp = nc.dram_tensor("out", [128, 4], F32, kind="ExternalOutput")

    with tile.TileContext(nc) as tc:
        with (
            tc.tile_pool(name="sb1", bufs=1) as sb1,
            tc.tile_pool(name="sb2", bufs=2) as sb2,
            tc.tile_pool(name="ps_m", bufs=3, space="PSUM") as ps_m,
            tc.tile_pool(name="ps_z", bufs=3, space="PSUM") as ps_z,
            tc.tile_pool(name="ps_w", bufs=2, space="PSUM") as ps_w,
        ):
            _emit(nc, tc, sb1, sb2, ps_m, ps_z, ps_w, xpk, hpack,
                  outp, hreg, hp_cols)

    nc.compile()
    return nc


def _emit(nc, tc, sb1, sb2, ps_m, ps_z, ps_w, xpk, hpack, outp,
          hreg, hp_cols):
    # ---- ONE wide input DMA (128 descriptors) + hp on the SWDGE queue ----
    xpa = sb1.tile([128, XPW], FP16, tag="xpa")
    hp = sb1.tile([128, hp_cols], FP16, tag="hp")
    nc.sync.dma_start(xpa[:], xpk[:])
    nc.gpsimd.dma_start(hp[:], hpack[:])
    va = xpa[:, VMO : VMO + VW]
    vb = xpa[0:94, XB + VMO : XB + VMO + VW]

    acc = sb1.tile([128, 4], F32, tag="acc")
    nc.vector.memset(acc[:], 0.0)
    warm = sb1.tile([1, 8], F32, tag="warm")
    nc.vector.memset(warm[:], 1.0)
    nc.scalar.activation(warm[:], warm[:], ActFn.Sqrt)
    nc.scalar.activation(warm[:], warm[:], ActFn.Square)

    def hblk(name, rows=128):
        lo, hi, off = hreg[name]
        return hp[0:rows, off : off + (hi - lo)], lo, hi

    copy_rr = [0]

    def copy_cast(dst_ap, src_ap, scale=None):
        # PSUM->SBUF copies alternate between DVE and ACT
        if scale is None and copy_rr[0] % 2 == 0:
            nc.vector.tensor_copy(dst_ap, src_ap)
        else:
            nc.scalar.activation(dst_ap, src_ap, ActFn.Copy,
                                 scale=1.0 if scale is None else scale)
        copy_rr[0] += 1

    # ---- zz0 / z12 at level 0 (DVE, from ximg directly) ------------------
    # zz0* layout: [*, 0:1024] = 121*x^2 (img1|img2), [*, 1024:1536] = x1*x2
    # band rows are host-permuted: z rows = xa[0:128] ++ xb[0:10]
    sa, sb_ = 128, 10
    zza = sb1.tile([sa, 1536], FP16, tag="zza")
    zzb = sb1.tile([sb_, 1536], FP16, tag="zzb")
    for (zt, c0, rows) in ((zza, 0, sa), (zzb, XB, sb_)):
        nc.scalar.activation(
            zt[0:rows, 0:1024], xpa[0:rows, c0 : c0 + 1024],
            ActFn.Square, scale=11.0
        )
        nc.vector.tensor_tensor(
            zt[0:rows, 1024:1536], xpa[0:rows, c0 : c0 + 512],
            xpa[0:rows, c0 + 512 : c0 + 1024], AluOp.mult,
        )

    # ---- stage 1: vertical everything (stationary = x chunks) ------------
    # vT_sb [128, 8, 448]: chunk gc (0-3 img1, 4-7 img2), cols per VOFF
    vt = sb1.tile([128, 8 * VW], FP16, tag="vt")
    vt3 = vt[:].rearrange("p (c v) -> p c v", v=VW)
    for gc in range(8):
        vps = ps_w.tile([128, VW], F32, tag="work", name=f"v{gc}")
        nc.tensor.matmul(vps[:], xpa[:, 128 * gc : 128 * gc + 128], va,
                         start=True, stop=False)
        nc.tensor.matmul(vps[:], xpa[0:94, XB + 128 * gc : XB + 128 * gc + 128],
                         vb, start=False, stop=True)
        copy_cast(vt3[:, gc, :], vps[:])

    # ---- z0 pass A': vTz (stationary = zz chunks, mobile = avz0) ---------
    vtz_ps = [ps_w.tile([128, 512], F32, tag="work", name=f"vtz{q}") for q in range(3)]
    for q in range(3):
        for k in range(4):
            nc.tensor.matmul(
                vtz_ps[q][:, 128 * k : 128 * k + 128],
                zza[:, 512 * q + 128 * k : 512 * q + 128 * k + 128],
                xpa[0:sa, AZO : AZO + 128], start=True, stop=False,
            )
            nc.tensor.matmul(
                vtz_ps[q][:, 128 * k : 128 * k + 128],
                zzb[:, 512 * q + 128 * k : 512 * q + 128 * k + 128],
                xpa[0:sb_, AZO + 128 : AZO + 256], start=False, stop=True,
            )
    vtz = sb1.tile([128, 1536], FP16, tag="vtz")
    for q in range(3):
        copy_cast(vtz[:, 512 * q : 512 * q + 512], vtz_ps[q][:])

    # ---- stage 2a: m-maps (stationary = vT slices, mobile = Bh0/BH_l) ----
    # md merged: img I occupies partitions 64*I..64*I+64
    m0_ps = [ps_m.tile([128, 512], F32, tag="m", name=f"m0_{I}") for I in range(2)]
    md_ps = ps_m.tile([128, 448], F32, tag="m", name="md")
    for I in range(2):
        for c in range(4):
            gc = 4 * I + c
            b0, lo, hi = hblk(f"bh0_{c}")
            nc.tensor.matmul(m0_ps[I][:, lo:hi], vt3[:, gc, 0:128], b0,
                             start=(c == 0), stop=(c == 3))
        for l in (1, 2, 3):
            for c in range(4):
                gc = 4 * I + c
                bb, lo, hi = hblk(f"BH{l}_{c}")
                nc.tensor.matmul(
                    md_ps[64 * I : 64 * I + R[l], DOFF[l] + lo : DOFF[l] + hi],
                    vt3[:, gc, VOFF[3 + l] : VOFF[3 + l] + R[l]], bb,
                    start=(c == 0), stop=(c == 3),
                )

    # ---- stage 2b: images xt_l (stationary = vT V-slices, mobile = Hm) ---
    # packed per level: [NKP_l, 2*H_l] (img1 cols 0:H_l, img2 H_l:2H_l)
    xt_ps = {
        1: ps_w.tile([74, 512], F32, tag="work", name="xt1"),
        2: ps_w.tile([42, 256], F32, tag="work", name="xt2"),
        3: ps_w.tile([26, 128], F32, tag="work", name="xt3"),
    }
    for l in (1, 2, 3):
        wl = H[l]  # per-image width
        for I in range(2):
            for c in range(4):
                gc = 4 * I + c
                hm, lo, hi = hblk(f"hm{l}_{c}")
                nc.tensor.matmul(
                    xt_ps[l][0 : NKZ[l], I * wl + lo : I * wl + hi],
                    vt3[:, gc, VOFF[l] : VOFF[l] + NKZ[l]], hm,
                    start=(c == 0), stop=(c == 3),
                )
    xt_sb = {
        1: sb1.tile([74, 512], FP16, tag="xt1s", name="xt1s"),
        2: sb1.tile([42, 256], FP16, tag="xt2s", name="xt2s"),
        3: sb1.tile([26, 128], FP16, tag="xt3s", name="xt3s"),
    }
    for l in (1, 2, 3):
        copy_cast(xt_sb[l][:], xt_ps[l][:])

    # ---- z0 pass B': zbox0 (stationary = vTz chunks, mobile = bh0) -------
    zb0_ps = [ps_z.tile([128, 512], F32, tag="zb", name=f"zb0_{q}") for q in range(3)]
    for q in range(3):
        for k in range(4):
            b0, lo, hi = hblk(f"bh0_{k}")
            nc.tensor.matmul(
                zb0_ps[q][:, lo:hi],
                vtz[:, 512 * q + 128 * k : 512 * q + 128 * k + 128], b0,
                start=(k == 0), stop=(k == 3),
            )

    # ---- deep zz / z12 (DVE from xt_sb) ----------------------------------
    # zzd_l layout: [rows, 0:2*wl] = 121*xt^2 (img1|img2), [*, 2wl:3wl] = xt1*xt2
    zzd = {}
    for l in (1, 2, 3):
        rows = NKZ[l]
        wl = H[l]
        zt = sb1.tile([rows, 3 * wl], FP16, tag=f"zzd{l}")
        nc.scalar.activation(
            zt[:, 0 : 2 * wl], xt_sb[l][:, :], ActFn.Square, scale=11.0
        )
        nc.gpsimd.tensor_tensor(
            zt[:, 2 * wl : 3 * wl], xt_sb[l][:, 0:wl], xt_sb[l][:, wl : 2 * wl],
            AluOp.mult,
        )
        zzd[l] = zt

    # ---- POINTWISE L0 phase 1: drain the PSUM maps into sig tiles --------
    S2 = SCL * SCL
    m1a, m2a = m0_ps[0][:, :], m0_ps[1][:, :]
    r11a, r22a, r12a = zb0_ps[0][:, :], zb0_ps[1][:, :], zb0_ps[2][:, :]
    m2s = sb2.tile([128, 512], FP16, tag="am2s")
    q1 = sb2.tile([128, 512], FP16, tag="aq1")
    q2 = sb2.tile([128, 512], FP16, tag="aq2")
    q12 = sb2.tile([128, 512], FP16, tag="aq12")
    sig1 = sb2.tile([128, 512], FP16, tag="as1")
    sig2 = sb2.tile([128, 512], FP16, tag="as2")
    sig12 = sb2.tile([128, 512], FP16, tag="as12")
    nc.scalar.activation(m2s[:], m2a, ActFn.Copy, scale=SCL)
    nc.scalar.activation(q1[:], m1a, ActFn.Square, scale=SCL)
    nc.vector.scalar_tensor_tensor(
        q2[:], m2s[:], 1.0, m2s[:], AluOp.mult, AluOp.mult
    )
    nc.vector.scalar_tensor_tensor(
        q12[:], m1a, SCL, m2s[:], AluOp.mult, AluOp.mult
    )
    nc.vector.scalar_tensor_tensor(
        sig1[:], r11a, S2, q1[:], AluOp.mult, AluOp.subtract
    )
    nc.vector.scalar_tensor_tensor(
        sig2[:], r22a, S2, q2[:], AluOp.mult, AluOp.subtract
    )
    nc.vector.scalar_tensor_tensor(
        sig12[:], r12a, 121.0 * S2, q12[:], AluOp.mult, AluOp.subtract
    )
    # ---- deep z pass A': vTzd (stationary = zzd chunks, mobile = avz_l) --
    # packed: L1 rects (q,k) -> [128, 64] at 64*(2q+k); L2 (q) -> [128,32] at
    # 384+32q; total [128, 480]; L3 (q) -> [64,16] in vtzd_b [64, 48]
    AZD = {1: (256, 74, 64), 2: (320, 42, 32), 3: (352, 26, 16)}
    vtzd_a = ps_z.tile([128, 480], F32, tag="zb", name="vtzd_a")
    vtzd_b = ps_w.tile([64, 48], F32, tag="work", name="vtzd_b")
    for l in (1, 2, 3):
        azo, rows, rl = AZD[l]
        wl = H[l] // 2
        nch = H[l] // 128 if H[l] >= 128 else 1
        cw = min(128, H[l])
        for q in range(3):
            for k in range(nch):
                if l == 1:
                    dst = vtzd_a[:, 64 * (2 * q + k) : 64 * (2 * q + k) + 64]
                elif l == 2:
                    dst = vtzd_a[:, 384 + 32 * q : 384 + 32 * q + 32]
                else:
                    dst = vtzd_b[:, 16 * q : 16 * q + 16]
                nc.tensor.matmul(
                    dst,
                    zzd[l][0:rows, cw * (nch * q + k) : cw * (nch * q + k) + cw],
                    xpa[0:rows, AZO + azo : AZO + azo + rl],
                    start=True, stop=True,
                )
    vtzd_s = sb1.tile([128, 480], FP16, tag="vtzd_s")
    vtzd_sb3 = sb1.tile([64, 48], FP16, tag="vtzd_s3")
    copy_cast(vtzd_s[:], vtzd_a[:])
    copy_cast(vtzd_sb3[:], vtzd_b[:])

    # ---- deep z pass B': zboxd (stationary = vTzd, mobile = bh_l) --------
    # merged: q=0 at partitions 0:64 and q=1 at 64:128 of zbdA; q=2 in zbdB
    zbdA = ps_z.tile([128, 448], F32, tag="zb", name="zbdA")
    zbdB = ps_z.tile([64, 448], F32, tag="zb", name="zbdB")
    for q in range(3):
        t, p0 = (zbdA, 64 * q) if q < 2 else (zbdB, 0)
        for k in range(2):  # L1
            b1, lo, hi = hblk(f"bh1_{k}")
            nc.tensor.matmul(
                t[p0 : p0 + 64, lo:hi],
                vtzd_s[:, 64 * (2 * q + k) : 64 * (2 * q + k) + 64], b1,
                start=(k == 0), stop=(k == 1),
            )
        b2, lo, hi = hblk("bh2")
        nc.tensor.matmul(
            t[p0 : p0 + 32, 256 + lo : 256 + hi],
            vtzd_s[:, 384 + 32 * q : 384 + 32 * q + 32], b2,
            start=True, stop=True,
        )
        b3, lo, hi = hblk("bh3", rows=64)
        nc.tensor.matmul(
            t[p0 : p0 + 16, 384 + lo : 384 + hi],
            vtzd_sb3[:, 16 * q : 16 * q + 16], b3,
            start=True, stop=True,
        )

    # ---- POINTWISE L0 phase 2 (pure SBUF chain) --------------------------
    pp0 = sb2.tile([128, 512], F32, tag="app")
    inv0 = sb2.tile([128, 512], F32, tag="ainv")
    rr0 = sb2.tile([128, 512], FP16, tag="arr")
    cs0 = sb2.tile([128, 512], FP16, tag="acs")
    nc.gpsimd.tensor_tensor(pp0[:], sig1[:], sig2[:], AluOp.mult)
    nc.vector.reciprocal_approx_fast(inv0[:], pp0[:])
    nc.scalar.activation(rr0[:], inv0[:], ActFn.Sqrt)
    nc.vector.scalar_tensor_tensor(
        cs0[:], sig12[:], 1.0, rr0[:], AluOp.mult, AluOp.mult,
        accum_out=acc[0:128, 0:1],
    )

    # ---- POINTWISE deep (per image; base-0 outputs via single-input ops) -
    m2sd = sb2.tile([64, 448], FP16, tag="dm2s")
    q1d = sb2.tile([64, 448], FP16, tag="dq1")
    q2d = sb2.tile([64, 448], FP16, tag="dq2")
    q12d = sb2.tile([64, 448], FP16, tag="dq12")
    sig1d = sb2.tile([64, 448], FP16, tag="ds1")
    sig2d = sb2.tile([64, 448], FP16, tag="ds2")
    s12d = sb2.tile([64, 448], FP16, tag="ds12")
    ppd = sb2.tile([64, 448], F32, tag="dpp")
    invd = sb2.tile([64, 448], F32, tag="dinv")
    rrd = sb2.tile([64, 448], FP16, tag="drr")
    csd = sb2.tile([64, 448], FP16, tag="dcs")
    nc.scalar.activation(m2sd[:], md_ps[64:128, :], ActFn.Copy, scale=SCL)
    nc.scalar.activation(q1d[:], md_ps[0:64, :], ActFn.Square, scale=SCL)
    nc.vector.scalar_tensor_tensor(
        q2d[:], m2sd[:], 1.0, m2sd[:], AluOp.mult, AluOp.mult
    )
    nc.vector.scalar_tensor_tensor(
        q12d[:], md_ps[0:64, :], SCL, m2sd[:], AluOp.mult, AluOp.mult
    )
    nc.vector.scalar_tensor_tensor(
        sig1d[:], zbdA[0:64, :], S2, q1d[:], AluOp.mult, AluOp.subtract
    )
    nc.vector.scalar_tensor_tensor(
        sig2d[:], zbdA[64:128, :], S2, q2d[:], AluOp.mult, AluOp.subtract
    )
    nc.vector.scalar_tensor_tensor(
        s12d[:], zbdB[0:64, :], 121.0 * S2, q12d[:], AluOp.mult, AluOp.subtract
    )
    nc.vector.tensor_tensor(ppd[:], sig1d[:], sig2d[:], AluOp.mult)
    nc.vector.reciprocal_approx_fast(invd[:], ppd[:])
    nc.scalar.activation(rrd[:], invd[:], ActFn.Sqrt)
    for (lv, pr, clo, chi) in ((1, 64, 0, 256), (2, 32, 256, 384),
                               (3, 16, 384, 448)):
        nc.vector.scalar_tensor_tensor(
            csd[0:pr, clo:chi], s12d[0:pr, clo:chi], 1.0,
            rrd[0:pr, clo:chi], AluOp.mult, AluOp.mult,
            accum_out=acc[0:pr, lv : lv + 1],
        )

    nc.sync.dma_start(outp[:], acc[:])


# ----------------------------------------------------------------------------
# public entry point
# ----------------------------------------------------------------------------
_CACHE = {}


def _get():
    if "nc" not in _CACHE:
        hp, hreg = _build_h()
        cores = [_build_v(i) for i in range(4)]
        _CACHE["hp"], _CACHE["hreg"] = hp, hreg
        _CACHE["cores"] = cores
        _CACHE["nc"] = build_program(hreg, hp.shape[1])
    return _CACHE["nc"]


def kernel(img1, img2, _run_kwargs=None):
    img1 = np.asarray(img1, np.float32)
    img2 = np.asarray(img2, np.float32)
    nc = _get()
    hp = _CACHE["hp"]
    in_maps = []
    for c in range(8):
        b, i = c // 4, c % 4
        vmat, azp, perm = _CACHE["cores"][i]
        in_maps.append({
            "xpk": _xpack(img1, img2, b, i, vmat, azp, perm),
            "hpack": hp,
        })
    res = run_bass_kernel_spmd(nc, in_maps, list(range(8)), **(_run_kwargs or {}))
    total = 0.0
    for l in range(4):
        s = 0.0
        for c in range(8):
            s += float(np.sum(res.results[c]["out"][0 : R[l], l].astype(np.float64)))
        mean_c = s / (2.0 * H[l] * H[l])
        total += PYR_W[l] * (2.0 - 2.0 * mean_c)
    out = np.float32(total)
    if _run_kwargs:
        return out, res
    return out
